# revision 17
# baseline (speedup 1.0000x reference)
"""Batched 2048-point complex DFT on 8 Trainium2 NeuronCores.

z = (x_r + i x_i) @ (W_r + i W_i) for x [8192, 2048] fp32, W the 2048x2048
DFT matrix.  Data-parallel: batch sharded 8 ways (1024 rows/core), weights
replicated (and recomputed host-side from the analytic DFT form).

Two device implementations:
  - "fft": 2-stage Cooley-Tukey factorization N = 128*16. Per stage the
    small DFT matrices sit block-diagonal in the PE's stationary operand,
    so PE work drops ~8x vs the dense matmul. Matmul operands are
    float32r (fp32 bits, PE-internal rounding, 1 cycle/row) -> ~1.5e-4
    rel err.
  - "direct": dense 4-matmul reference implementation (fallback).

Pipeline of "fft" per core, per batch-quarter (256 rows):
  1. PE-transpose x tiles into FFT-permuted layout xt[t] (p = 16j'+n2).
  2. Stage 1: per tile t, psum = W1[t].T @ xt[t] (block-diag radix-16 DFT
     with twiddles folded in), evict to y1 staging -> DRAM.
  3. Corner turn: gather stage-2 input tiles y2[t2] from DRAM (the FFT
     all-to-all; SBUF APs cannot cross partitions, DRAM APs can).
  4. Stage 2: data-stationary matmul  psum[b, 0:128|128:256] =
     y2re.T @ [G2re|G2im] + y2im.T @ [-G2im|G2re]  — output lands already
     in [batch, freq] orientation, no transpose-back needed.
  5. Scatter-evict psum columns k = 16*k1 + t2 into z staging, DMA out.
"""

import os
import sys

sys.path.insert(0, "/opt/trn_rl_repo")
os.environ.setdefault("MYCRO_LOCAL_CACHE", "1")
os.environ.setdefault("JAX_PLATFORMS", "axon,cpu")

import numpy as np

import concourse.bass as bass
import concourse.bacc as bacc
import concourse.mybir as mybir
from concourse import tile
from concourse import bass_utils

F32 = mybir.dt.float32
F32R = mybir.dt.float32r
MM_DT = F32R

N = 2048          # DFT size
B_CORE = 1024     # batch rows per core (8192 / 8)
N_CORES = 8
P = 128

SCHEME = os.environ.get("DFT_SCHEME", "fft")

# ---------------------------------------------------------------- tables ---

def _fft_tables():
    """Stage tables for N = 128*16: n = n1 + 128*n2, k = 16*k1 + k2."""
    w1re = np.zeros((128, 16, 128), np.float64)
    w1im = np.zeros((128, 16, 128), np.float64)
    for t in range(16):
        for jp in range(8):
            n1 = 8 * t + jp
            n2 = np.arange(16)[:, None]
            k2 = np.arange(16)[None, :]
            ang = -2.0 * np.pi * (((n1 + 128 * n2) * k2) % N) / N
            # stage-1 input partition p = 8*n2 + jp, output m = 8*k2 + jp
            w1re[jp::8, t, jp::8] = np.cos(ang)
            w1im[jp::8, t, jp::8] = np.sin(ang)
    n1 = np.arange(128)[:, None]
    k1 = np.arange(128)[None, :]
    ang2 = -2.0 * np.pi * ((n1 * k1) % 128) / 128
    g2re, g2im = np.cos(ang2), np.sin(ang2)
    f32 = np.float32
    return {
        "w1re": w1re.reshape(128, 16 * 128).astype(f32),
        "w1im": w1im.reshape(128, 16 * 128).astype(f32),
        "w1nim": (-w1im).reshape(128, 16 * 128).astype(f32),
        "g2a": np.concatenate([g2re, g2im], axis=1).astype(f32),
        "g2b": np.concatenate([-g2im, g2re], axis=1).astype(f32),
        "ident": np.eye(128, dtype=f32),
    }


# ------------------------------------------------------------ fft kernel ---

def build_fft_kernel(repeat=1):
    nc = bacc.Bacc("TRN2", target_bir_lowering=False, debug=False)

    xr_d = nc.dram_tensor("xr", (B_CORE, N), MM_DT, kind="ExternalInput")
    xi_d = nc.dram_tensor("xi", (B_CORE, N), MM_DT, kind="ExternalInput")
    w1re_d = nc.dram_tensor("w1re", (P, 16 * P), MM_DT, kind="ExternalInput")
    w1im_d = nc.dram_tensor("w1im", (P, 16 * P), MM_DT, kind="ExternalInput")
    w1nim_d = nc.dram_tensor("w1nim", (P, 16 * P), MM_DT, kind="ExternalInput")
    g2a_d = nc.dram_tensor("g2a", (P, 2 * P), MM_DT, kind="ExternalInput")
    g2b_d = nc.dram_tensor("g2b", (P, 2 * P), MM_DT, kind="ExternalInput")
    id_d = nc.dram_tensor("ident", (P, P), MM_DT, kind="ExternalInput")
    zr_d = nc.dram_tensor("zr", (B_CORE, N), F32, kind="ExternalOutput")
    zi_d = nc.dram_tensor("zi", (B_CORE, N), F32, kind="ExternalOutput")

    Q = 4            # batch quarters
    BQ = B_CORE // Q  # 256 rows
    NBT = BQ // P     # 2 b-tiles per quarter

    with tile.TileContext(nc) as tc:
        with (
            tc.tile_pool(name="const", bufs=1) as cp,
            tc.tile_pool(name="xstage", bufs=4) as xsp,
            tc.tile_pool(name="xt", bufs=1) as xtp,
            tc.tile_pool(name="y1s", bufs=3) as y1p,
            tc.tile_pool(name="y2", bufs=3) as y2p,
            tc.tile_pool(name="zstage", bufs=1) as zp,
            tc.tile_pool(name="y1d", bufs=2, space="DRAM") as ddp,
            tc.tile_pool(name="tpsum", bufs=2, space="PSUM") as tpp,
            tc.tile_pool(name="s1psum", bufs=2, space="PSUM") as s1p,
            tc.tile_pool(name="s2psum", bufs=2, space="PSUM") as s2p,
        ):
            ident = cp.tile([P, P], MM_DT)
            w1re = cp.tile([P, 16, P], MM_DT)
            w1im = cp.tile([P, 16, P], MM_DT)
            w1nim = cp.tile([P, 16, P], MM_DT)
            g2a = cp.tile([P, 2 * P], MM_DT)
            g2b = cp.tile([P, 2 * P], MM_DT)
            nc.sync.dma_start(ident[:], id_d.ap())
            nc.sync.dma_start(w1re[:], w1re_d.ap().rearrange("p (t m) -> p t m", t=16))
            nc.sync.dma_start(w1im[:], w1im_d.ap().rearrange("p (t m) -> p t m", t=16))
            nc.sync.dma_start(w1nim[:], w1nim_d.ap().rearrange("p (t m) -> p t m", t=16))
            nc.sync.dma_start(g2a[:], g2a_d.ap())
            nc.sync.dma_start(g2b[:], g2b_d.ap())

            def ev(i, dst, src):
                if i % 2 == 0:
                    nc.vector.tensor_copy(dst, src)
                else:
                    nc.scalar.copy(dst, src)

            import contextlib

            rep_ctx = (
                tc.For_i(0, repeat, 1) if repeat > 1 else contextlib.nullcontext()
            )
            with rep_ctx:
              for q in range(Q):
                c0 = q * BQ
                xtr = xtp.tile([P, 16, BQ], MM_DT, tag="xtr")
                xti = xtp.tile([P, 16, BQ], MM_DT, tag="xti")

                # --- load + transpose-in (PE transpose mode) ---
                tc.strict_bb_all_engine_barrier()
                for src_d, dst in ((xr_d, xtr), (xi_d, xti)):
                    stgs = []
                    for bt in range(NBT):
                        stg = xsp.tile([P, N], MM_DT, tag="xs")
                        # FFT-permuted column load: column c = 128*t + 8*n2 + j
                        # holds x[.., n] with n = 8*t + j + 128*n2.
                        # (DMA APs max 3 dims -> one DMA per t.)
                        src_v = src_d.ap()[
                            c0 + bt * P : c0 + (bt + 1) * P, :
                        ].rearrange("b (n2 t j) -> b t n2 j", n2=16, j=8)
                        for t in range(16):
                            nc.sync.dma_start(
                                stg[:, t * P : (t + 1) * P].rearrange(
                                    "b (n2 j) -> b n2 j", j=8
                                ),
                                src_v[:, t],
                            )
                        stgs.append(stg)
                    for t in range(16):
                        ps = tpp.tile([P, BQ], MM_DT, tag="tp")
                        for bt in range(NBT):
                            in_ = stgs[bt][:, t * P : (t + 1) * P]
                            nc.tensor.matmul(
                                ps[:, bt * P : (bt + 1) * P],
                                in_,
                                ident[:],
                                is_transpose=True,
                                start=(bt == 0),
                                stop=(bt == NBT - 1),
                            )
                        ev(t, dst[:, t, :], ps[:])
                tc.strict_bb_all_engine_barrier()

                # --- stage 1: block-diag radix-16 DFT + twiddles ---
                y1rd = ddp.tile([16 * P, BQ], MM_DT, tag="y1r")
                y1id = ddp.tile([16 * P, BQ], MM_DT, tag="y1i")
                for t in range(16):
                    psR = s1p.tile([P, BQ], F32, tag="s1r")
                    psI = s1p.tile([P, BQ], F32, tag="s1i")
                    nc.tensor.matmul(psR[:], w1re[:, t, :], xtr[:, t, :], start=True, stop=False)
                    nc.tensor.matmul(psR[:], w1nim[:, t, :], xti[:, t, :], start=False, stop=True)
                    nc.tensor.matmul(psI[:], w1re[:, t, :], xti[:, t, :], start=True, stop=False)
                    nc.tensor.matmul(psI[:], w1im[:, t, :], xtr[:, t, :], start=False, stop=True)
                    y1r = y1p.tile([P, BQ], MM_DT, tag="y1r")
                    y1i = y1p.tile([P, BQ], MM_DT, tag="y1i")
                    ev(t, y1r[:], psR[:])
                    ev(t + 1, y1i[:], psI[:])
                    nc.sync.dma_start(y1rd[t * P : (t + 1) * P, :], y1r[:])
                    nc.sync.dma_start(y1id[t * P : (t + 1) * P, :], y1i[:])

                # --- corner turn (via DRAM) + stage 2 + scatter ---
                zsts = [[None] * NBT for _ in range(2)]
                for pl in range(2):
                    for bt in range(NBT):
                        zst = zp.tile([P, N], F32, tag=f"z{pl}{bt}", name=f"zst{pl}{bt}")
                        zsts[pl][bt] = zst
                y1rd_v = y1rd[:].rearrange("(t q_) c -> t q_ c", q_=P)
                y1id_v = y1id[:].rearrange("(t q_) c -> t q_ c", q_=P)
                for t2 in range(16):
                    y2r = y2p.tile([P, BQ], MM_DT, tag="y2r")
                    y2i = y2p.tile([P, BQ], MM_DT, tag="y2i")
                    nc.sync.dma_start(y2r[:], y1rd_v[:, 8 * t2 : 8 * t2 + 8, :])
                    nc.sync.dma_start(y2i[:], y1id_v[:, 8 * t2 : 8 * t2 + 8, :])
                    for bt in range(NBT):
                        ps2 = s2p.tile([P, 2 * P], F32, tag="s2")
                        lr = y2r[:, bt * P : (bt + 1) * P]
                        li = y2i[:, bt * P : (bt + 1) * P]
                        nc.tensor.matmul(ps2[:], lr, g2a[:], start=True, stop=False)
                        nc.tensor.matmul(ps2[:], li, g2b[:], start=False, stop=True)
                        zr_v = zsts[0][bt][:].rearrange("p (k1 k2) -> p k1 k2", k2=16)
                        zi_v = zsts[1][bt][:].rearrange("p (k1 k2) -> p k1 k2", k2=16)
                        ev(t2, zr_v[:, :, t2], ps2[:, 0:P])
                        ev(t2 + 1, zi_v[:, :, t2], ps2[:, P : 2 * P])
                for bt in range(NBT):
                    r0 = c0 + bt * P
                    nc.sync.dma_start(zr_d.ap()[r0 : r0 + P, :], zsts[0][bt][:])
                    nc.sync.dma_start(zi_d.ap()[r0 : r0 + P, :], zsts[1][bt][:])

    nc.compile()
    return nc


# --------------------------------------------------------- direct kernel ---

def build_direct_kernel():
    nc = bacc.Bacc("TRN2", target_bir_lowering=False, debug=False)

    KT = N // P
    FB = 256
    NFB = N // FB
    BH = 512
    NM = BH // P

    xr_d = nc.dram_tensor("xr", (B_CORE, N), MM_DT, kind="ExternalInput")
    xi_d = nc.dram_tensor("xi", (B_CORE, N), MM_DT, kind="ExternalInput")
    wr_d = nc.dram_tensor("wr", (N, N), MM_DT, kind="ExternalInput")
    wi_d = nc.dram_tensor("wi", (N, N), MM_DT, kind="ExternalInput")
    id_d = nc.dram_tensor("ident", (P, P), MM_DT, kind="ExternalInput")
    zr_d = nc.dram_tensor("zr", (B_CORE, N), F32, kind="ExternalOutput")
    zi_d = nc.dram_tensor("zi", (B_CORE, N), F32, kind="ExternalOutput")

    wr_t = wr_d.ap().rearrange("(kt p) n -> p kt n", p=P)
    wi_t = wi_d.ap().rearrange("(kt p) n -> p kt n", p=P)

    with tile.TileContext(nc) as tc:
        with (
            tc.tile_pool(name="const", bufs=1) as const_pool,
            tc.tile_pool(name="xstage", bufs=3) as xstage_pool,
            tc.tile_pool(name="xT", bufs=1) as xt_pool,
            tc.tile_pool(name="w", bufs=2) as w_pool,
            tc.tile_pool(name="zstage", bufs=4) as z_pool,
            tc.tile_pool(name="tpsum", bufs=2, space="PSUM") as tpsum_pool,
            tc.tile_pool(name="mpsum", bufs=2, space="PSUM") as mpsum_pool,
        ):
            ident = const_pool.tile([P, P], MM_DT)
            nc.sync.dma_start(ident[:], id_d.ap())

            for half in range(2):
                b0 = half * BH
                xTr = xt_pool.tile([P, KT, BH], MM_DT, tag="xTr")
                xTi = xt_pool.tile([P, KT, BH], MM_DT, tag="xTi")

                for plane, (src, dst) in enumerate(((xr_d, xTr), (xi_d, xTi))):
                    for bt in range(NM):
                        stg = xstage_pool.tile([P, N], MM_DT, tag="xstg")
                        nc.sync.dma_start(
                            stg[:], src.ap()[b0 + bt * P : b0 + (bt + 1) * P, :]
                        )
                        for kt in range(KT):
                            ps = tpsum_pool.tile([P, P], MM_DT, tag="tp")
                            nc.tensor.transpose(
                                ps[:], stg[:, kt * P : (kt + 1) * P], ident[:]
                            )
                            nc.vector.tensor_copy(
                                dst[:, kt, bt * P : (bt + 1) * P], ps[:]
                            )

                tc.strict_bb_all_engine_barrier()

                for fb in range(NFB):
                    f0 = fb * FB
                    wrt = w_pool.tile([P, KT, FB], MM_DT, tag="wr")
                    wit = w_pool.tile([P, KT, FB], MM_DT, tag="wi")
                    wnit = w_pool.tile([P, KT, FB], MM_DT, tag="wni")
                    nc.sync.dma_start(wrt[:], wr_t[:, :, f0 : f0 + FB])
                    nc.sync.dma_start(wit[:], wi_t[:, :, f0 : f0 + FB])
                    nc.vector.tensor_scalar_mul(wnit[:], wit[:], -1.0)

                    for m in range(NM):
                        ps_r = mpsum_pool.tile([P, FB], F32, tag="ps_r")
                        ps_i = mpsum_pool.tile([P, FB], F32, tag="ps_i")
                        for kt in range(KT):
                            st = kt == 0
                            lr = xTr[:, kt, m * P : (m + 1) * P]
                            li = xTi[:, kt, m * P : (m + 1) * P]
                            wr_k = wrt[:, kt, :]
                            nc.tensor.matmul(ps_r[:], lr, wr_k, start=st, stop=False)
                            nc.tensor.matmul(ps_i[:], li, wr_k, start=st, stop=False)
                        for kt in range(KT):
                            sp = kt == KT - 1
                            lr = xTr[:, kt, m * P : (m + 1) * P]
                            li = xTi[:, kt, m * P : (m + 1) * P]
                            nc.tensor.matmul(ps_r[:], li, wnit[:, kt, :], start=False, stop=sp)
                            nc.tensor.matmul(ps_i[:], lr, wit[:, kt, :], start=False, stop=sp)

                        zr_s = z_pool.tile([P, FB], F32, tag="zr_s")
                        zi_s = z_pool.tile([P, FB], F32, tag="zi_s")
                        nc.vector.tensor_copy(zr_s[:], ps_r[:])
                        nc.vector.tensor_copy(zi_s[:], ps_i[:])
                        r0 = b0 + m * P
                        nc.sync.dma_start(zr_d.ap()[r0 : r0 + P, f0 : f0 + FB], zr_s[:])
                        nc.sync.dma_start(zi_d.ap()[r0 : r0 + P, f0 : f0 + FB], zi_s[:])

    nc.compile()
    return nc


# ---------------------------------------------------------------- driver ---

_NC_CACHE = {}


def _get_nc(scheme=None):
    scheme = scheme or SCHEME
    if scheme not in _NC_CACHE:
        _NC_CACHE[scheme] = (
            build_fft_kernel() if scheme == "fft" else build_direct_kernel()
        )
    return _NC_CACHE[scheme]


def make_in_maps(x_real, x_imag, W_real, W_imag, scheme=None):
    scheme = scheme or SCHEME
    x_real = np.asarray(x_real, dtype=np.float32)
    x_imag = np.asarray(x_imag, dtype=np.float32)
    tabs = _fft_tables()
    in_maps = []
    for c in range(N_CORES):
        sl = slice(c * B_CORE, (c + 1) * B_CORE)
        m = {
            "xr": np.ascontiguousarray(x_real[sl]),
            "xi": np.ascontiguousarray(x_imag[sl]),
            "ident": tabs["ident"],
        }
        if scheme == "fft":
            for k in ("w1re", "w1im", "w1nim", "g2a", "g2b"):
                m[k] = tabs[k]
        else:
            m["wr"] = np.ascontiguousarray(np.asarray(W_real, dtype=np.float32))
            m["wi"] = np.ascontiguousarray(np.asarray(W_imag, dtype=np.float32))
        in_maps.append(m)
    return in_maps


def kernel(x_real, x_imag, W_real, W_imag):
    nc = _get_nc()
    in_maps = make_in_maps(x_real, x_imag, W_real, W_imag)
    res = bass_utils.run_bass_kernel_spmd(nc, in_maps, core_ids=list(range(N_CORES)))
    zr = np.concatenate([res.results[c]["zr"] for c in range(N_CORES)], axis=0)
    zi = np.concatenate([res.results[c]["zi"] for c in range(N_CORES)], axis=0)
    return zr, zi
